# revision 24
# baseline (speedup 1.0000x reference)
import sys
sys.path.insert(0, "/opt/trn_rl_repo")
import numpy as np

import concourse.bass as bass
import concourse.bacc as bacc
import concourse.mybir as mybir
import concourse.tile as tile
from concourse.bass import AP, IndirectOffsetOnAxis
from concourse.bass_utils import run_bass_kernel_spmd

F32 = mybir.dt.float32
BF16 = mybir.dt.bfloat16
I32 = mybir.dt.int32
Tanh = mybir.ActivationFunctionType.Tanh
Sigm = mybir.ActivationFunctionType.Sigmoid
Exp = mybir.ActivationFunctionType.Exp
AbsF = mybir.ActivationFunctionType.Abs
Identity = mybir.ActivationFunctionType.Identity
ADD = mybir.AluOpType.add
SUB = mybir.AluOpType.subtract
MUL = mybir.AluOpType.mult
MAXOP = mybir.AluOpType.max
AXX = mybir.AxisListType.X

B, T, Tq, S, G, Hd, GW, V = 128, 512, 32, 64, 300, 256, 512, 159
NCORE = 8
Bc = B // NCORE   # 16
H3 = 3 * Hd       # 768
NJ = 6
GP = 384          # padded G (ones row at 300)
XA4 = 512         # padded answer-x rows (4 chunks of 128)


def bf(x):
    import jax.numpy as jnp
    return np.asarray(jnp.asarray(np.asarray(x), jnp.bfloat16))


def bcast_s(ap, s=S):
    """[128, n] AP -> [128, n, s] AP with step-0 broadcast inner dim."""
    return AP(ap.tensor, ap.offset, tuple(list(ap.ap) + [[0, s]]))


def build_program():
    nc = bacc.Bacc("TRN2", target_bir_lowering=False, debug=False,
                   num_devices=NCORE)
    D = {}
    def din(name, shape, dt):
        D[name] = nc.dram_tensor(name, list(shape), dt, kind="ExternalInput")

    din("xt", (3, 128, T * Bc), BF16)
    din("xtq", (3, 128, Tq * Bc), BF16)
    din("wih_i", (GP, H3), BF16); din("whh_i", (Hd, H3), BF16)
    din("wih_q", (GP, H3), BF16); din("whh_q", (Hd, H3), BF16)
    din("bnhh_i", (2, 128), BF16); din("bnhh_q", (2, 128), BF16)
    din("c2", (2, 32), BF16)
    # phase-2 (memory/answer cells, old convention)
    din("wih_m", (Hd, H3), BF16); din("whh_m", (Hd, H3), BF16)
    din("wih_a", (XA4, H3), BF16); din("whh_a", (Hd, H3), BF16)
    din("bM_m", (NJ, 128), BF16); din("bM_a", (NJ, 128), BF16)
    din("bn2_m", (2, 128), BF16); din("bn2_a", (2, 128), BF16)
    din("c6", (NJ, NJ * Bc), BF16); din("c01", (2, 2 * Bc), BF16)
    din("w1t", (1792, GW), BF16)
    din("w8r", (1, GW), BF16); din("w9r", (1, GW), BF16)
    din("zwt", (Hd, Hd), BF16); din("gw2t", (128, 4), BF16)
    din("gb1v", (128, 4), F32); din("gb2h", (1, 1), F32)
    din("wat", (Hd, V), BF16)
    din("eye", (128, 128), BF16)
    din("fidx", (128, 16), I32)
    din("qix", (Bc, 2), I32)
    pred_d = nc.dram_tensor("pred", [Bc, V], F32, kind="ExternalOutput")
    NBLK = (T + 1) * 32 // 128 + 1      # 129 transpose blocks (last is 32 cols)
    NBLKQ = (Tq + 1) * 32 // 128 + 1    # 9
    hall_d = nc.dram_tensor("hall", [NBLK * 128, 128], BF16,
                            kind="Internal")
    hallq_d = nc.dram_tensor("hallq", [NBLKQ * 128, 128], BF16,
                             kind="Internal")

    with tile.TileContext(nc) as tc:
        with tc.tile_pool(name="w", bufs=1) as pw:
            def load2(name, rows, cols):
                nch = (rows + 127) // 128
                t_ = pw.tile([128, nch * cols], D[name].dtype, tag=name)
                for c in range(nch):
                    r0, r1 = c * 128, min((c + 1) * 128, rows)
                    nc.sync.dma_start(t_[0:r1 - r0, c * cols:c * cols + cols],
                                      D[name].ap()[r0:r1, :])
                return t_

            wih_i = load2("wih_i", GP, H3); whh_i = load2("whh_i", Hd, H3)
            wih_q = load2("wih_q", GP, H3); whh_q = load2("whh_q", Hd, H3)
            bnhh_i = load2("bnhh_i", 2, 128); bnhh_q = load2("bnhh_q", 2, 128)
            c2 = load2("c2", 2, 32)
            wih_m = load2("wih_m", Hd, H3); whh_m = load2("whh_m", Hd, H3)
            wih_a = load2("wih_a", XA4, H3); whh_a = load2("whh_a", Hd, H3)
            bM_m = load2("bM_m", NJ, 128); bM_a = load2("bM_a", NJ, 128)
            bn2_m = load2("bn2_m", 2, 128); bn2_a = load2("bn2_a", 2, 128)
            c6 = load2("c6", NJ, NJ * Bc); c01 = load2("c01", 2, 2 * Bc)
            w1t = load2("w1t", 1792, GW)
            w8r = load2("w8r", 1, GW); w9r = load2("w9r", 1, GW)
            zwt = load2("zwt", Hd, Hd); gw2t = load2("gw2t", 128, 4)
            gb1v = load2("gb1v", 128, 4); gb2h = load2("gb2h", 1, 1)
            wat = load2("wat", Hd, V)
            eye = load2("eye", 128, 128)
            fidx = load2("fidx", 128, 16)
            qix = load2("qix", Bc, 2)
            ones1 = pw.tile([128, 1], BF16, tag="ones1")
            nc.vector.memset(ones1[:, :], 1.0)

            # ---------------- old-style GRU cell (phase 2 only) ------------
            def cell_common(psc, P, n2_ap, git_rz, h_chunks, h_full, h_dst):
                if git_rz is not None:
                    nc.vector.tensor_tensor(out=P[:, 0:64], in0=P[:, 0:64],
                                            in1=git_rz, op=ADD)
                trz = psc.tile([128, 64], F32, tag="trz")
                nc.scalar.activation(trz[:, :], P[:, 0:64], Tanh, scale=0.5)
                u = psc.tile([128, 32], F32, tag="u")
                nc.vector.scalar_tensor_tensor(
                    out=u[:, :], in0=trz[:, 0:32], scalar=1.0,
                    in1=P[:, 64:96], op0=ADD, op1=MUL)
                w_ = psc.tile([128, 32], F32, tag="wt")
                nc.vector.tensor_tensor(out=w_[:, :], in0=u[:, :],
                                        in1=n2_ap, op=ADD)
                tn = psc.tile([128, 32], F32, tag="tn")
                nc.scalar.activation(tn[:, :], w_[:, :], Tanh, scale=0.5)
                d_ = psc.tile([128, 32], F32, tag="d")
                nc.vector.tensor_tensor(out=d_[:, :], in0=h_full,
                                        in1=tn[:, :], op=SUB)
                e_ = psc.tile([128, 32], F32, tag="e")
                nc.vector.scalar_tensor_tensor(
                    out=e_[:, :], in0=trz[:, 32:64], scalar=1.0, in1=d_[:, :],
                    op0=ADD, op1=MUL)
                nc.vector.scalar_tensor_tensor(
                    out=h_dst, in0=e_[:, :], scalar=0.5, in1=tn[:, :],
                    op0=MUL, op1=ADD)

            def h_mms(P, whh, bM, h_chunks, extra):
                nc.tensor.matmul(P[:, 0:96], lhsT=bM[0:NJ, 0:128],
                                 rhs=c6[0:NJ, :], start=True, stop=False,
                                 skip_group_check=True)
                ops = []
                for j in range(NJ):
                    for c in range(2):
                        ops.append((whh[:, c * H3 + j * 128:c * H3 + (j + 1) * 128],
                                    h_chunks[c], j))
                ops += extra
                for k, (lhsT, rhs, j) in enumerate(ops):
                    nc.tensor.matmul(P[:, j * Bc:(j + 1) * Bc], lhsT=lhsT,
                                     rhs=rhs, start=False,
                                     stop=(k == len(ops) - 1),
                                     skip_group_check=True)

            def cell_livex(pps, psc, xchunks, h_chunks, h_full, h_dst,
                           wih, whh, bM, bn2):
                P = pps.tile([128, 96], F32, tag="gruP")
                extra = []
                for (rhs, cc, kk) in xchunks:
                    for j in range(4):
                        extra.append(
                            (wih[0:kk, cc * H3 + j * 128:cc * H3 + (j + 1) * 128],
                             rhs, j))
                h_mms(P, whh, bM, h_chunks, extra)
                PB = pps.tile([128, 32], F32, tag="gruP", name="PB")
                nc.tensor.matmul(PB[:, :], lhsT=bn2[0:2, 0:128],
                                 rhs=c01[0:2, :], start=True, stop=False,
                                 skip_group_check=True)
                k = 0
                nmm = len(xchunks) * 2
                for (rhs, cc, kk) in xchunks:
                    for j in range(4, 6):
                        k += 1
                        nc.tensor.matmul(
                            PB[:, (j - 4) * Bc:(j - 3) * Bc],
                            lhsT=wih[0:kk, cc * H3 + j * 128:cc * H3 + (j + 1) * 128],
                            rhs=rhs, start=False, stop=(k == nmm),
                            skip_group_check=True)
                cell_common(psc, P, PB[:, :], None, h_chunks, h_full, h_dst)

            # ============ phase 1: fully unrolled GRU recurrences ===========
            with tc.tile_pool(name="st", bufs=1) as pst, \
                 tc.tile_pool(name="ps1", bufs=1, space="PSUM") as pps, \
                 tc.tile_pool(name="sc1", bufs=1) as psc:

                HCOLS = ((T + 1) * 32 + 127) // 128 * 128      # 16512
                HQCOLS = ((Tq + 1) * 32 + 127) // 128 * 128    # 1152
                hallT = pst.tile([128, HCOLS], BF16, tag="hallT")
                hallqT = pst.tile([128, HQCOLS], BF16, tag="hallqT")
                nc.vector.memset(hallT[:, 0:32], 0.0)
                nc.vector.memset(hallqT[:, 0:32], 0.0)
                nc.vector.memset(hallT[:, (T + 1) * 32:], 0.0)
                nc.vector.memset(hallqT[:, (Tq + 1) * 32:], 0.0)

                psRZ = pps.tile([128, 2048], F32, tag="psRZ")
                psN = pps.tile([128, 512], F32, tag="psN")
                psG = [pps.tile([128, 512], F32, tag=f"psG{i}",
                                name=f"psG{i}")
                       for i in range(2)]
                xts = [psc.tile([128, 3 * 256], BF16, tag=f"xt{i}",
                                name=f"xt{i}") for i in range(3)]
                stgs = [psc.tile([128, 128], BF16, tag=f"stg{i}",
                                 name=f"stg{i}") for i in range(4)]
                # per-step SBUF rings
                RING = 3
                trzs = [psc.tile([128, 64], F32, tag=f"trz{i}",
                                 name=f"trz{i}") for i in range(RING)]
                us = [psc.tile([128, 32], F32, tag=f"u{i}", name=f"u{i}")
                      for i in range(RING)]
                ws = [psc.tile([128, 32], F32, tag=f"w{i}", name=f"w{i}")
                      for i in range(RING)]
                tns = [psc.tile([128, 32], F32, tag=f"tn{i}", name=f"tn{i}")
                       for i in range(RING)]
                zhs = [psc.tile([128, 32], F32, tag=f"zh{i}", name=f"zh{i}")
                       for i in range(RING)]
                hms = [psc.tile([128, 32], F32, tag=f"hm{i}", name=f"hm{i}")
                       for i in range(RING)]
                qs = [psc.tile([128, 32], F32, tag=f"q{i}", name=f"q{i}")
                      for i in range(RING)]
                # question-GRU dedicated resources (bank 7 + SBUF git)
                psQ = pps.tile([128, 512], F32, tag="psQ")
                qxt = psc.tile([128, 3 * Tq * Bc], BF16, tag="qxt")
                gitq = psc.tile([128, 6 * Tq * Bc], BF16, tag="gitq")
                stgq = psc.tile([128, 128], BF16, tag="stgq")
                qring = [dict(
                    trz=psc.tile([128, 64], F32, tag=f"qtrz{i}",
                                 name=f"qtrz{i}"),
                    u=psc.tile([128, 32], F32, tag=f"qu{i}", name=f"qu{i}"),
                    w=psc.tile([128, 32], F32, tag=f"qw{i}", name=f"qw{i}"),
                    tn=psc.tile([128, 32], F32, tag=f"qtn{i}",
                                name=f"qtn{i}"),
                    zh=psc.tile([128, 32], F32, tag=f"qzh{i}",
                                name=f"qzh{i}"),
                    hm=psc.tile([128, 32], F32, tag=f"qhm{i}",
                                name=f"qhm{i}"),
                    q=psc.tile([128, 32], F32, tag=f"qq{i}", name=f"qq{i}"),
                ) for i in range(RING)]

                def rz_ap(half, tt):
                    t2 = psRZ[:, :]
                    return AP(t2.tensor, t2.offset + half * 256 + tt * 16,
                              (t2.ap[0], [512, 4], [1, 16]))

                def gin_ap(half, tt):
                    g2 = psG[half][:, :]
                    return AP(g2.tensor, g2.offset + tt * 16,
                              (g2.ap[0], [256, 2], [1, 16]))

                def v3(ap2d):
                    return ap2d.rearrange("p (c b) -> p c b", b=16)

                def emit_xproj_mms(wih, xtile, half):
                    """list of thunks: x-projection matmuls for one 16-step
                    group into psRZ halves (rz) and psG[half] (n)."""
                    thunks = []
                    for j in range(4):
                        for gc in range(3):
                            def f(j=j, gc=gc):
                                nc.tensor.matmul(
                                    psRZ[:, j * 512 + half * 256:
                                         j * 512 + half * 256 + 256],
                                    lhsT=wih[:, gc * H3 + j * 128:
                                             gc * H3 + (j + 1) * 128],
                                    rhs=xtile[:, gc * 256:(gc + 1) * 256],
                                    start=(gc == 0), stop=False,
                                    skip_group_check=True)
                            thunks.append(f)
                    for j in range(4, 6):
                        for gc in range(3):
                            def f(j=j, gc=gc):
                                nc.tensor.matmul(
                                    psG[half][:, (j - 4) * 256:
                                              (j - 4) * 256 + 256],
                                    lhsT=wih[:, gc * H3 + j * 128:
                                             gc * H3 + (j + 1) * 128],
                                    rhs=xtile[:, gc * 256:(gc + 1) * 256],
                                    start=(gc == 0), stop=(gc == 2),
                                    skip_group_check=True)
                            thunks.append(f)
                    return thunks

                def emit_step(t, whh, bnhh, hallT, half):
                    tt = t % 16
                    r = t % RING
                    h_ap = hallT[:, t * 32:(t + 1) * 32]
                    hc = [hallT[:, t * 32:t * 32 + 16],
                          hallT[:, t * 32 + 16:(t + 1) * 32]]
                    # PE: rz blocks
                    for j in range(4):
                        for c in range(2):
                            nc.tensor.matmul(
                                psRZ[:, j * 512 + half * 256 + tt * 16:
                                     j * 512 + half * 256 + tt * 16 + 16],
                                lhsT=whh[:, c * H3 + j * 128:
                                         c * H3 + (j + 1) * 128],
                                rhs=hc[c], start=False, stop=(c == 1),
                                skip_group_check=True)
                    # ACT: r | zb = sigmoid(rz psum, z-cols pre-negated).
                    # Emitted BEFORE the n-gate matmuls: the sigmoid's PE
                    # semaphore wait is a cumulative engine counter, so this
                    # ordering lets it fire right after the rz matmuls.
                    trz = trzs[r]
                    nc.scalar.activation(
                        trz[:, :].rearrange("p (a b) -> p a b", b=16),
                        rz_ap(half, tt), Sigm)
                    # PE: n blocks (bias + 4 mms)
                    nslot = psN[:, tt * 32:(tt + 1) * 32]
                    nc.tensor.matmul(nslot, lhsT=bnhh[0:2, 0:128],
                                     rhs=c2[0:2, 0:32], start=True,
                                     stop=False, skip_group_check=True)
                    for c in range(2):
                        for j in range(4, 6):
                            nc.tensor.matmul(
                                psN[:, tt * 32 + (j - 4) * 16:
                                    tt * 32 + (j - 4) * 16 + 16],
                                lhsT=whh[:, c * H3 + j * 128:
                                         c * H3 + (j + 1) * 128],
                                rhs=hc[c], start=False,
                                stop=(c == 1 and j == 5),
                                skip_group_check=True)
                    # V: u = r * P_n ; w = u + gin (critical path; keep V
                    # clear of the off-path zh/hm, which go to gpsimd)
                    u, w_, tn = us[r], ws[r], tns[r]
                    nc.vector.tensor_tensor(out=u[:, :], in0=trz[:, 0:32],
                                            in1=nslot, op=MUL)
                    nc.vector.tensor_tensor(out=v3(w_[:, :]),
                                            in0=v3(u[:, :]),
                                            in1=gin_ap(half, tt), op=ADD)
                    # ACT: tn = tanh(w)
                    nc.scalar.activation(tn[:, :], w_[:, :], Tanh)
                    # V (overlapped with tanh): zh = zb*h ; hm = h - zh
                    zh, hm, q_ = zhs[r], hms[r], qs[r]
                    nc.vector.tensor_tensor(out=zh[:, :], in0=trz[:, 32:64],
                                            in1=h_ap, op=MUL)
                    nc.vector.tensor_tensor(out=hm[:, :], in0=h_ap,
                                            in1=zh[:, :], op=SUB)
                    # V: q = zb*tn ; h' = hm + q
                    nc.vector.tensor_tensor(out=q_[:, :], in0=trz[:, 32:64],
                                            in1=tn[:, :], op=MUL)
                    nc.vector.tensor_tensor(
                        out=hallT[:, (t + 1) * 32:(t + 2) * 32],
                        in0=hm[:, :], in1=q_[:, :], op=ADD)

                def emit_store(hallT, hdram, k, cols):
                    # transpose on the Sync DGE queue, copy-out on the idle
                    # GpSimd queue; ring of 4 keeps the pipeline deep enough
                    # that stores track the recurrence instead of draining
                    # in a serial tail after it.
                    stg = stgs[k % 4]
                    nc.sync.dma_start_transpose(
                        stg[0:cols, 0:128],
                        hallT[:, k * 128:k * 128 + cols])
                    nc.gpsimd.dma_start(hdram.ap()[k * 128:k * 128 + cols, :],
                                        stg[0:cols, 0:128])

                def emit_gru(T_, xt_d, wih, whh, bnhh_t, hallT_t, hdram,
                             total_cols, qhook=None):
                    NG = T_ // 16
                    def dma_group(g):
                        if g >= NG:
                            return
                        xt_tile = xts[g % 3]
                        for gc in range(3):
                            nc.sync.dma_start(
                                xt_tile[:, gc * 256:(gc + 1) * 256],
                                xt_d.ap()[gc, :, g * 256:(g + 1) * 256])
                    dma_group(0); dma_group(1)
                    nblk = (total_cols + 127) // 128
                    kdone = 0
                    for g in range(NG):
                        # x-proj burst at the group boundary: start=True
                        # clears has_written for the WHOLE bank, so it must
                        # not interleave with the previous group's in-flight
                        # step accumulations in the same banks. PE executes
                        # matmuls in order, so boundary emission is safe.
                        for f in emit_xproj_mms(wih, xts[g % 3], g % 2):
                            f()
                        for tt in range(16):
                            t = g * 16 + tt
                            if tt == 0:
                                dma_group(g + 2)
                            emit_step(t, whh, bnhh_t, hallT_t, g % 2)
                            if qhook is not None and t < Tq:
                                qhook(t)
                            # batched history stores
                            while (kdone < nblk and
                                   (kdone * 128 + 128 <= (t + 2) * 32
                                    or t == T_ - 1)):
                                cols = min(128, total_cols - kdone * 128)
                                emit_store(hallT_t, hdram, kdone, cols)
                                kdone += 1

                # --- question GRU: git staged through bank 7 into SBUF,
                # its 32 steps interleave with the input GRU's first 32 ---
                TB = Tq * Bc   # 512
                for gc in range(3):
                    nc.sync.dma_start(qxt[:, gc * TB:(gc + 1) * TB],
                                      D["xtq"].ap()[gc, :, :])
                for j in range(NJ):
                    for gc in range(3):
                        nc.tensor.matmul(
                            psQ[:, :],
                            lhsT=wih_q[:, gc * H3 + j * 128:
                                       gc * H3 + (j + 1) * 128],
                            rhs=qxt[:, gc * TB:(gc + 1) * TB],
                            start=(gc == 0), stop=(gc == 2),
                            skip_group_check=True)
                    nc.scalar.copy(gitq[:, j * TB:(j + 1) * TB], psQ[:, :])

                def gitq_ap(t, j0, nj):
                    g2 = gitq[:, :]
                    return AP(g2.tensor, g2.offset + j0 * TB + t * 16,
                              (g2.ap[0], [TB, nj], [1, 16]))

                QR = 5  # bank-7 psum ring: 5 slots of 96 cols
                def emit_qstep(t):
                    r = t % RING
                    qr = qring[r]
                    slot = psQ[:, (t % QR) * 96:(t % QR) * 96 + 96]
                    h_ap = hallqT[:, t * 32:(t + 1) * 32]
                    hc = [hallqT[:, t * 32:t * 32 + 16],
                          hallqT[:, t * 32 + 16:(t + 1) * 32]]
                    # bias mm first: its start=True clears bank 7 (safe: all
                    # other slots' accumulations completed earlier in PE order)
                    nc.tensor.matmul(slot[:, 64:96], lhsT=bnhh_q[0:2, 0:128],
                                     rhs=c2[0:2, 0:32], start=True,
                                     stop=False, skip_group_check=True)
                    # git_rz into cols 0:64 via identity (overwrite: bank was
                    # just cleared so accumulate-writes act as overwrite)
                    nc.tensor.matmul(slot[:, 0:64], lhsT=eye[:, :],
                                     rhs=gitq_ap(t, 0, 4), start=False,
                                     stop=False, skip_group_check=True)
                    for j in range(4):
                        for c in range(2):
                            nc.tensor.matmul(
                                slot[:, j * 16:(j + 1) * 16],
                                lhsT=whh_q[:, c * H3 + j * 128:
                                           c * H3 + (j + 1) * 128],
                                rhs=hc[c], start=False, stop=(c == 1),
                                skip_group_check=True)
                    trz = qr["trz"]
                    nc.scalar.activation(trz[:, :], slot[:, 0:64], Sigm)
                    for c in range(2):
                        for j in range(4, 6):
                            nc.tensor.matmul(
                                slot[:, 64 + (j - 4) * 16:64 + (j - 3) * 16],
                                lhsT=whh_q[:, c * H3 + j * 128:
                                           c * H3 + (j + 1) * 128],
                                rhs=hc[c], start=False,
                                stop=(c == 1 and j == 5),
                                skip_group_check=True)
                    u, w_, tn = qr["u"], qr["w"], qr["tn"]
                    nc.vector.tensor_tensor(out=u[:, :], in0=trz[:, 0:32],
                                            in1=slot[:, 64:96], op=MUL)
                    nc.vector.tensor_tensor(out=v3(w_[:, :]),
                                            in0=v3(u[:, :]),
                                            in1=gitq_ap(t, 4, 2), op=ADD)
                    nc.scalar.activation(tn[:, :], w_[:, :], Tanh)
                    zh, hm, q_ = qr["zh"], qr["hm"], qr["q"]
                    nc.vector.tensor_tensor(out=zh[:, :], in0=trz[:, 32:64],
                                            in1=h_ap, op=MUL)
                    nc.vector.tensor_tensor(out=hm[:, :], in0=h_ap,
                                            in1=zh[:, :], op=SUB)
                    nc.vector.tensor_tensor(out=q_[:, :], in0=trz[:, 32:64],
                                            in1=tn[:, :], op=MUL)
                    nc.vector.tensor_tensor(
                        out=hallqT[:, (t + 1) * 32:(t + 2) * 32],
                        in0=hm[:, :], in1=q_[:, :], op=ADD)
                    # batched hallq stores
                    k = (t - 2) // 4
                    if t >= 2 and (t - 2) % 4 == 0:
                        nc.sync.dma_start_transpose(
                            stgq[:, :], hallqT[:, k * 128:(k + 1) * 128])
                        nc.sync.dma_start(
                            hallq_d.ap()[k * 128:(k + 1) * 128, :],
                            stgq[:, :])
                    if t == Tq - 1:
                        for k in range((Tq * 32 + 64) // 128, HQCOLS // 128):
                            nc.sync.dma_start_transpose(
                                stgq[:, :], hallqT[:, k * 128:(k + 1) * 128])
                            nc.sync.dma_start(
                                hallq_d.ap()[k * 128:(k + 1) * 128, :],
                                stgq[:, :])

                emit_gru(T, D["xt"], wih_i, whh_i, bnhh_i, hallT,
                         hall_d, HCOLS, qhook=emit_qstep)

            # ============ phase 2: gather + attention + answer ==========
            with tc.tile_pool(name="att", bufs=1) as pa, \
                 tc.tile_pool(name="ps2", bufs=2, space="PSUM") as pps2, \
                 tc.tile_pool(name="sc2", bufs=2) as ps2:

                facts = pa.tile([128, 8 * Hd], BF16, tag="facts")
                for i in range(8):
                    for c in range(2):
                        nc.gpsimd.indirect_dma_start(
                            out=facts[:, i * Hd + c * 128:i * Hd + (c + 1) * 128],
                            out_offset=None,
                            in_=hall_d.ap(),
                            in_offset=IndirectOffsetOnAxis(
                                ap=fidx[:, i * 2 + c:i * 2 + c + 1], axis=0))
                qsb = pa.tile([16, Hd], BF16, tag="qsb")
                for c in range(2):
                    nc.gpsimd.indirect_dma_start(
                        out=qsb[:, c * 128:(c + 1) * 128], out_offset=None,
                        in_=hallq_d.ap(),
                        in_offset=IndirectOffsetOnAxis(
                            ap=qix[0:Bc, c:c + 1], axis=0))
                ct = [pa.tile([128, 1024], BF16, tag=f"ct{c}", name=f"ct{c}")
                      for c in range(2)]
                for i in range(8):
                    for c in range(2):
                        nc.sync.dma_start_transpose(
                            ct[c][:, i * 128:(i + 1) * 128],
                            facts[:, i * Hd + c * 128:i * Hd + (c + 1) * 128])
                qt = pa.tile([128, 32], BF16, tag="qt")
                for c in range(2):
                    ptq = pps2.tile([128, 16], BF16, tag="ps_a")
                    nc.tensor.transpose(ptq[:, :],
                                        qsb[:, c * 128:(c + 1) * 128],
                                        eye[0:16, 0:16])
                    nc.scalar.copy(qt[:, c * 16:(c + 1) * 16], ptq[:, :])

                def w1(block_chunk, c, j):
                    col = (block_chunk + c) * GW + j * 128
                    return w1t[:, col:col + 128]

                cwt = [pa.tile([128, 1024], BF16, tag=f"cwt{c}", name=f"cwt{c}")
                       for c in range(2)]
                for co in range(2):
                    for ns in range(2):
                        PC = pps2.tile([128, 512], F32, tag="ps_a")
                        for hc in range(2):
                            nc.tensor.matmul(
                                PC[:, :],
                                lhsT=zwt[:, hc * Hd + co * 128:hc * Hd + (co + 1) * 128],
                                rhs=ct[hc][:, ns * 512:(ns + 1) * 512],
                                start=(hc == 0), stop=(hc == 1))
                        nc.scalar.copy(cwt[co][:, ns * 512:(ns + 1) * 512],
                                       PC[:, :])

                def mul3(dst_bf, a2d, v16):
                    nc.vector.tensor_tensor(
                        out=dst_bf.rearrange("p (b s) -> p b s", s=S),
                        in0=a2d.rearrange("p (b s) -> p b s", s=S),
                        in1=bcast_s(v16), op=MUL)

                def abs_diff(dst_bf, a2d, v16, psc_):
                    tmp = psc_.tile([128, 1024], F32, tag="adtmp")
                    nc.vector.tensor_tensor(
                        out=tmp[:, :].rearrange("p (b s) -> p b s", s=S),
                        in0=a2d.rearrange("p (b s) -> p b s", s=S),
                        in1=bcast_s(v16), op=SUB)
                    # |x| = max(-x, x) in one V op (keeps ACT free)
                    nc.vector.scalar_tensor_tensor(
                        out=dst_bf, in0=tmp[:, :], scalar=-1.0,
                        in1=tmp[:, :], op0=MUL, op1=MAXOP)

                def colreduce_dot(vt):
                    tmps = []
                    for c in range(2):
                        tmp = ps2.tile([128, 1024], BF16, tag=f"drtmp{c}",
                                       name=f"drtmp{c}")
                        mul3(tmp[:, :], cwt[c][:, :],
                             vt[:, c * 16:(c + 1) * 16])
                        tmps.append(tmp)
                    drow = ps2.tile([1, 1024], BF16, tag="drow")
                    for ns in range(2):
                        dps = pps2.tile([1, 512], F32, tag="ps_b",
                                        name=f"dps{ns}")
                        for c in range(2):
                            nc.tensor.matmul(
                                dps[:, :], lhsT=ones1[:, :],
                                rhs=tmps[c][:, ns * 512:(ns + 1) * 512],
                                start=(c == 0), stop=(c == 1))
                        nc.scalar.copy(drow[:, ns * 512:(ns + 1) * 512],
                                       dps[:, :])
                    return drow

                def small_proj(vt, block_chunk):
                    out = ps2.tile([128, 4 * 16], F32, tag="sproj")
                    for j in range(4):
                        pp = pps2.tile([128, 16], F32, tag="ps_a")
                        for c in range(2):
                            nc.tensor.matmul(
                                pp[:, :], lhsT=w1(block_chunk, c, j),
                                rhs=vt[:, c * 16:(c + 1) * 16],
                                start=(c == 0), stop=(c == 1))
                        nc.scalar.copy(out[:, j * 16:(j + 1) * 16], pp[:, :])
                    return out

                # hop-invariant terms
                cq = [pa.tile([128, 1024], BF16, tag=f"cq{c}", name=f"cq{c}")
                      for c in range(2)]
                dq = [pa.tile([128, 1024], BF16, tag=f"dq{c}", name=f"dq{c}")
                      for c in range(2)]
                for c in range(2):
                    mul3(cq[c][:, :], ct[c][:, :], qt[:, c * 16:(c + 1) * 16])
                    abs_diff(dq[c][:, :], ct[c][:, :],
                             qt[:, c * 16:(c + 1) * 16], ps2)
                qproj = small_proj(qt, 2)
                d2row = colreduce_dot(qt)
                h1base = [pa.tile([128, 1024], BF16, tag=f"h1b{j}", name=f"h1b{j}")
                          for j in range(4)]
                for j in range(4):
                    for ns in range(2):
                        PH = pps2.tile([128, 512], F32, tag="ps_a")
                        first = True
                        for c in range(2):
                            sl = slice(ns * 512, (ns + 1) * 512)
                            nc.tensor.matmul(PH[:, :], lhsT=w1(0, c, j),
                                             rhs=ct[c][:, sl], start=first,
                                             stop=False, skip_group_check=True)
                            first = False
                            nc.tensor.matmul(PH[:, :], lhsT=w1(8, c, j),
                                             rhs=cq[c][:, sl], start=False,
                                             stop=False, skip_group_check=True)
                            nc.tensor.matmul(PH[:, :], lhsT=w1(12, c, j),
                                             rhs=dq[c][:, sl], start=False,
                                             stop=False, skip_group_check=True)
                        nc.tensor.matmul(
                            PH[:, :], lhsT=w9r[0:1, j * 128:(j + 1) * 128],
                            rhs=d2row[:, ns * 512:(ns + 1) * 512],
                            start=False, stop=True, skip_group_check=True)
                        qpb = bcast_s(qproj[:, j * 16 + ns * 8:j * 16 + (ns + 1) * 8])
                        nc.vector.scalar_tensor_tensor(
                            out=h1base[j][:, ns * 512:(ns + 1) * 512].rearrange(
                                "p (b s) -> p b s", s=S),
                            in0=PH[:, :].rearrange("p (b s) -> p b s", s=S),
                            scalar=1.0, in1=qpb, op0=MUL, op1=ADD)

                # hops
                mt = pa.tile([128, 32], BF16, tag="mt0")
                nc.gpsimd.tensor_copy(mt[:, :], qt[:, :])
                for hop in range(3):
                    cm = [ps2.tile([128, 1024], BF16, tag=f"cm{c}", name=f"cm{c}")
                          for c in range(2)]
                    dm = [ps2.tile([128, 1024], BF16, tag=f"dm{c}", name=f"dm{c}")
                          for c in range(2)]
                    for c in range(2):
                        mul3(cm[c][:, :], ct[c][:, :],
                             mt[:, c * 16:(c + 1) * 16])
                        abs_diff(dm[c][:, :], ct[c][:, :],
                                 mt[:, c * 16:(c + 1) * 16], ps2)
                    d1row = colreduce_dot(mt)
                    h1m = [ps2.tile([128, 1024], BF16, tag=f"h1m{j}", name=f"h1m{j}")
                           for j in range(4)]
                    for j in range(4):
                        for ns in range(2):
                            PH = pps2.tile([128, 512], F32, tag="ps_a")
                            first = True
                            for c in range(2):
                                sl = slice(ns * 512, (ns + 1) * 512)
                                nc.tensor.matmul(PH[:, :], lhsT=w1(6, c, j),
                                                 rhs=cm[c][:, sl], start=first,
                                                 stop=False,
                                                 skip_group_check=True)
                                first = False
                                nc.tensor.matmul(PH[:, :], lhsT=w1(10, c, j),
                                                 rhs=dm[c][:, sl], start=False,
                                                 stop=False,
                                                 skip_group_check=True)
                                # fold the per-(j,b) mproj broadcast-add into
                                # the PSUM group via an s-broadcast rhs
                                nc.tensor.matmul(
                                    PH[:, :].rearrange("p (b s) -> p b s", s=S),
                                    lhsT=w1(4, c, j),
                                    rhs=bcast_s(mt[:, c * 16 + ns * 8:
                                                   c * 16 + ns * 8 + 8]),
                                    start=False, stop=False,
                                    skip_group_check=True)
                            nc.tensor.matmul(
                                PH[:, :], lhsT=w8r[0:1, j * 128:(j + 1) * 128],
                                rhs=d1row[:, ns * 512:(ns + 1) * 512],
                                start=False, stop=False, skip_group_check=True)
                            # fold the h1base add via identity matmul
                            nc.tensor.matmul(
                                PH[:, :], lhsT=eye[:, :],
                                rhs=h1base[j][:, ns * 512:(ns + 1) * 512],
                                start=False, stop=True, skip_group_check=True)
                            nc.scalar.activation(
                                h1m[j][:, ns * 512:(ns + 1) * 512], PH[:, :],
                                Tanh, bias=gb1v[:, j:j + 1])
                    tat = ps2.tile([1, 1024], F32, tag="tat")
                    for ns in range(2):
                        gps = pps2.tile([1, 512], F32, tag="ps_b",
                                        name=f"gps{ns}")
                        for j in range(4):
                            nc.tensor.matmul(
                                gps[:, :],
                                lhsT=gw2t[:, j:j + 1],
                                rhs=h1m[j][:, ns * 512:(ns + 1) * 512],
                                start=(j == 0), stop=(j == 3))
                        nc.scalar.activation(tat[:, ns * 512:(ns + 1) * 512],
                                             gps[:, :], Tanh,
                                             scale=0.5, bias=gb2h[0:1, 0:1])
                    esc = ps2.tile([1, 1024], F32, tag="esc")
                    nc.scalar.activation(esc[:, :], tat[:, :], Exp, scale=0.5)
                    ssum = ps2.tile([1, 16], F32, tag="ssum")
                    nc.vector.tensor_reduce(
                        out=ssum[:, :],
                        in_=esc[:, :].rearrange("p (b s) -> p b s", s=S),
                        axis=AXX, op=ADD)
                    rs = ps2.tile([1, 16], F32, tag="rs")
                    nc.vector.reciprocal(rs[:, :], ssum[:, :])
                    soft = ps2.tile([1, 1024], F32, tag="soft")
                    nc.vector.tensor_tensor(
                        out=soft[:, :].rearrange("p (b s) -> p b s", s=S),
                        in0=esc[:, :].rearrange("p (b s) -> p b s", s=S),
                        in1=bcast_s(rs[:, :]), op=MUL)
                    softb = ps2.tile([128, 1024], F32, tag="softb")
                    nc.gpsimd.partition_broadcast(softb[:, :], soft[:, :])
                    htc = ps2.tile([128, 32], F32, tag="htc")
                    for c in range(2):
                        tmp = ps2.tile([128, 1024], F32, tag="htctmp")
                        nc.vector.tensor_tensor(out=tmp[:, :],
                                                in0=ct[c][:, :],
                                                in1=softb[:, :], op=MUL)
                        nc.vector.tensor_reduce(
                            out=htc[:, c * 16:(c + 1) * 16],
                            in_=tmp[:, :].rearrange("p (b s) -> p b s", s=S),
                            axis=AXX, op=ADD)
                    htcb = ps2.tile([128, 32], BF16, tag="htcb")
                    nc.gpsimd.tensor_copy(htcb[:, :], htc[:, :])
                    mt2 = pa.tile([128, 32], BF16, tag=f"mt{hop + 1}")
                    xch = [(htcb[:, 0:16], 0, 128), (htcb[:, 16:32], 1, 128)]
                    cell_livex(pps2, ps2, xch,
                               [mt[:, 0:16], mt[:, 16:32]], mt[:, :],
                               mt2[:, :], wih_m, whh_m, bM_m, bn2_m)
                    mt = mt2

                # answer module
                def wa_softmax(state_t, nm):
                    yp = pps2.tile([16, V], F32, tag="ps_a")
                    for c in range(2):
                        nc.tensor.matmul(yp[:, :],
                                         lhsT=state_t[:, c * 16:(c + 1) * 16],
                                         rhs=wat[:, c * V:(c + 1) * V],
                                         start=(c == 0), stop=(c == 1))
                    mx = ps2.tile([16, 1], F32, tag="mx")
                    nc.vector.tensor_reduce(out=mx[:, :], in_=yp[:, :],
                                            axis=AXX, op=MAXOP)
                    mxn = ps2.tile([16, 1], F32, tag="mxn")
                    nc.vector.tensor_scalar_mul(mxn[:, :], mx[:, :], -1.0)
                    ey = ps2.tile([16, V], F32, tag="ey")
                    nc.scalar.activation(ey[:, :], yp[:, :], Exp,
                                         bias=mxn[:, :])
                    sy = ps2.tile([16, 1], F32, tag="sy")
                    nc.vector.tensor_reduce(out=sy[:, :], in_=ey[:, :],
                                            axis=AXX, op=ADD)
                    ry = ps2.tile([16, 1], F32, tag="ry")
                    nc.vector.reciprocal(ry[:, :], sy[:, :])
                    yt = ps2.tile([16, V], F32, tag=nm)
                    nc.vector.tensor_scalar(out=yt[:, :], in0=ey[:, :],
                                            scalar1=ry[:, :], scalar2=None,
                                            op0=MUL)
                    return yt

                yt0 = wa_softmax(mt, "yt0")
                ytb = ps2.tile([16, V], BF16, tag="ytb")
                nc.gpsimd.tensor_copy(ytb[:, :], yt0[:, :])
                ytt = pa.tile([128, 32], BF16, tag="ytt")
                nc.vector.memset(ytt[:, :], 0.0)
                for c in range(2):
                    c0, c1 = c * 128, min((c + 1) * 128, V)
                    ptp = pps2.tile([128, 16], BF16, tag="ps_a")
                    nc.tensor.transpose(ptp[0:c1 - c0, :], ytb[:, c0:c1],
                                        eye[0:16, 0:16])
                    nc.scalar.copy(ytt[0:c1 - c0, c * 16:(c + 1) * 16],
                                   ptp[0:c1 - c0, :])
                at1 = pa.tile([128, 32], BF16, tag="at1")
                xch = [(ytt[:, 0:16], 0, 128), (ytt[:, 16:32], 1, 128),
                       (qt[:, 0:16], 2, 128), (qt[:, 16:32], 3, 128)]
                cell_livex(pps2, ps2, xch, [mt[:, 0:16], mt[:, 16:32]],
                           mt[:, :], at1[:, :], wih_a, whh_a, bM_a, bn2_a)
                yt1 = wa_softmax(at1, "yt1")
                nc.sync.dma_start(pred_d.ap()[:, :], yt1[:, :])

    nc.compile()
    return nc


_NC_CACHE = None


def _prep_gru_old(Wih, Whh, bih, bhh, xrows, xrows_pad):
    Wih = np.asarray(Wih, np.float32); Whh = np.asarray(Whh, np.float32)
    bih = np.asarray(bih, np.float32); bhh = np.asarray(bhh, np.float32)
    WihT = np.zeros((xrows_pad, H3), np.float32)
    WihT[0:xrows] = Wih.T
    WihT[:, 2 * Hd:] *= 2.0
    WhhT = Whh.T.copy()
    bM = np.zeros((NJ, 128), np.float32)
    bn2 = np.zeros((2, 128), np.float32)
    for j in range(NJ):
        blk = slice(j * 128, (j + 1) * 128)
        if j < 4:
            bM[j] = (bih + bhh)[blk]
        else:
            bn2[j - 4] = 2.0 * bih[blk]
            bM[j] = bhh[blk]
    return bf(WihT), bf(WhhT), bf(bM), bf(bn2)


def _prep_gru_new(Wih, Whh, bih, bhh):
    """sigmoid-native prep: bias folded into x-proj ones-row, z-gate
    columns negated so sigmoid yields (1-z)."""
    Wih = np.asarray(Wih, np.float32); Whh = np.asarray(Whh, np.float32)
    bih = np.asarray(bih, np.float32); bhh = np.asarray(bhh, np.float32)
    WihT = np.zeros((GP, H3), np.float32)
    WihT[0:G] = Wih.T
    brow = np.concatenate([(bih + bhh)[0:2 * Hd], bih[2 * Hd:]])
    WihT[G] = brow
    WihT[:, Hd:2 * Hd] *= -1.0
    WhhT = Whh.T.copy()
    WhhT[:, Hd:2 * Hd] *= -1.0
    bnhh = np.zeros((2, 128), np.float32)
    bnhh[0] = bhh[2 * Hd:2 * Hd + 128]
    bnhh[1] = bhh[2 * Hd + 128:]
    return bf(WihT), bf(WhhT), bf(bnhh)


def _xt_host(xc, T_):
    """[Bc, T_, G] f32 -> [3, 128, T_*Bc] bf16 with ones row at G."""
    xr = np.transpose(np.asarray(xc, np.float32), (2, 1, 0)).reshape(
        G, T_ * Bc)
    xt = np.zeros((3, 128, T_ * Bc), np.float32)
    xt[0] = xr[0:128]
    xt[1] = xr[128:256]
    xt[2, 0:G - 256] = xr[256:G]
    xt[2, G - 256] = 1.0
    return bf(xt)


def kernel(**inputs):
    global _NC_CACHE
    x = np.asarray(inputs["input"], np.float32)
    Q = np.asarray(inputs["Q"], np.float32)
    eos = np.asarray(inputs["EOS_idx"]).astype(np.int64)
    qlen = np.asarray(inputs["q_seq_len"]).astype(np.int64)

    wih_i, whh_i, bnhh_i = _prep_gru_new(
        inputs["Wih_i"], inputs["Whh_i"], inputs["bih_i"], inputs["bhh_i"])
    wih_q, whh_q, bnhh_q = _prep_gru_new(
        inputs["Wih_q"], inputs["Whh_q"], inputs["bih_q"], inputs["bhh_q"])
    wih_m, whh_m, bM_m, bn2_m = _prep_gru_old(
        inputs["Wih_m"], inputs["Whh_m"], inputs["bih_m"], inputs["bhh_m"],
        Hd, Hd)
    wih_a0, whh_a, bM_a, bn2_a = _prep_gru_old(
        inputs["Wih_a"], inputs["Whh_a"], inputs["bih_a"], inputs["bhh_a"],
        415, 415)
    # repack answer x-weights into 4x128-row chunks
    wa0 = np.asarray(wih_a0, np.float32)
    wih_a_pk = np.zeros((XA4, H3), np.float32)
    wih_a_pk[0:128] = wa0[0:128]
    wih_a_pk[128:159] = wa0[128:159]
    wih_a_pk[256:384] = wa0[159:287]
    wih_a_pk[384:512] = wa0[287:415]

    c2 = np.zeros((2, 32), np.float32)
    c2[0, 0:16] = 1.0; c2[1, 16:32] = 1.0

    gW1T = np.asarray(inputs["gW1"], np.float32).T.copy()   # [1794, 512]
    w1t = bf(gW1T[0:1792])
    w8r = bf(gW1T[1792:1793])
    w9r = bf(gW1T[1793:1794])
    zwt = bf(np.asarray(inputs["zW"], np.float32).T)
    gw2 = np.asarray(inputs["gW2"], np.float32).reshape(-1)
    gw2t = bf(gw2.reshape(4, 128).T)
    gb1v = np.asarray(inputs["gb1"], np.float32).reshape(4, 128).T.copy()
    gb2h = np.array([[0.5 * float(np.asarray(inputs["gb2"]).reshape(-1)[0])]],
                    np.float32)
    wat = bf(np.asarray(inputs["Wa"], np.float32).T)
    c6 = np.zeros((NJ, NJ * Bc), np.float32)
    for j in range(NJ):
        c6[j, j * Bc:(j + 1) * Bc] = 1.0
    c01 = np.zeros((2, 2 * Bc), np.float32)
    c01[0, 0:Bc] = 1.0; c01[1, Bc:2 * Bc] = 1.0
    eye = np.eye(128, dtype=np.float32)

    if _NC_CACHE is None:
        _NC_CACHE = build_program()
    nc = _NC_CACHE

    in_maps = []
    for core in range(NCORE):
        bsl = slice(core * Bc, (core + 1) * Bc)
        eos_c = eos[bsl]
        qlen_c = qlen[bsl]
        fidx = np.zeros((128, 16), np.int32)
        for i in range(8):
            for p in range(128):
                r = i * 128 + p
                b_, s_ = r // S, r % S
                t_ = int(eos_c[b_, s_])
                for c in range(2):
                    fidx[p, i * 2 + c] = (t_ + 1) * 32 + c * 16 + b_
        qidx = np.clip(qlen_c - 1, 0, Tq - 1).astype(np.int32)
        qix = np.stack([(qidx + 1) * 32 + c * 16 +
                        np.arange(Bc, dtype=np.int32)
                        for c in range(2)], axis=1).astype(np.int32)
        in_maps.append({
            "xt": _xt_host(x[bsl], T),
            "xtq": _xt_host(Q[bsl], Tq),
            "wih_i": wih_i, "whh_i": whh_i, "wih_q": wih_q, "whh_q": whh_q,
            "bnhh_i": bnhh_i, "bnhh_q": bnhh_q, "c2": bf(c2),
            "wih_m": wih_m, "whh_m": whh_m, "wih_a": bf(wih_a_pk),
            "whh_a": whh_a,
            "bM_m": bM_m, "bM_a": bM_a,
            "bn2_m": bn2_m, "bn2_a": bn2_a,
            "c6": bf(c6), "c01": bf(c01),
            "w1t": w1t, "w8r": w8r, "w9r": w9r, "zwt": zwt, "gw2t": gw2t,
            "gb1v": gb1v, "gb2h": gb2h, "wat": wat, "eye": bf(eye),
            "fidx": fidx, "qix": qix,
        })

    res = run_bass_kernel_spmd(nc, in_maps, core_ids=list(range(NCORE)))
    global LAST_RESULTS
    LAST_RESULTS = res
    out = np.concatenate([res.results[c]["pred"] for c in range(NCORE)],
                         axis=0)
    return out.astype(np.float32)


LAST_RESULTS = None
